# revision 11
# baseline (speedup 1.0000x reference)
"""Causal self-attention with RoPE for B=2, N=2048, D=2048, 16 heads,
distributed over 8 trn2 NeuronCores.

Sharding: core c = (b, g) with b = c // 4 (batch), g = c % 4 (head group of 4
heads).  Each core computes qkv projections + RoPE + causal attention for its
4 heads on its batch; per-head AllGathers ship each head's y^T to the other 3
cores of the batch while later heads still compute; each core then computes a
disjoint 512-column slice of the final o-projection.

v2 design notes (vs the fp32r two-half baseline):
- All matmul operands are bf16 (PSUM accumulation stays fp32).  bf16 enables
  FastWeightLoad (halves the per-matmul LDWEIGHTS cost that dominated the
  fp32r version), 2-4x DVE throughput, and half the DMA bytes.
- Stage A holds all 16 k-chunks of x^T in SBUF at once (bf16 makes it fit)
  and accumulates each qkv output tile across the full contraction in one
  PSUM bank: no half-pass SBUF adds, one PSUM->SBUF copy per tile, RoPE
  applied on the copy's output.
- Weights are host-packed so each logical stream is one large DMA.
- Softmax uses no max-subtraction (scores are O(1) by construction), masks
  multiply after exp, and row-normalization happens on y^T via a
  matmul-replicated reciprocal of the ones-matmul denominator.
"""

import numpy as np
from contextlib import ExitStack

import ml_dtypes

import concourse.bass as bass
import concourse.tile as tile
import concourse.mybir as mybir
from concourse.bass_utils import run_bass_kernel_spmd

F32 = mybir.dt.float32
F32R = mybir.dt.float32r
BF16 = mybir.dt.bfloat16

B = 2
N = 2048
D = 2048
H_TOT = 16
HD = 128  # head dim
HL = 4  # heads per core
N_CORES = 8
ROPE_BASE = 10000.0
INV_SQRT_HD = 1.0 / float(np.sqrt(HD))

NT = N // 512  # 4 n-tiles of 512
KC = D // 128  # 16 contraction chunks
ACT_COPY = mybir.ActivationFunctionType.Copy
ACT_EXP = mybir.ActivationFunctionType.Exp
ACT_LN = mybir.ActivationFunctionType.Ln


def split_multi_waits(nc, max_waits=1):
    """This container's walrus supports a single sync-wait per instruction;
    move extra waits onto preceding same-engine NoOps."""
    ctr = 0
    for f in nc.m.functions:
        for bb in f.blocks:
            new_list = []
            for inst in bb.instructions:
                si = inst.sync_info
                if si is not None and len(si.on_wait) > max_waits:
                    waits = list(si.on_wait)
                    for w in waits[:-max_waits]:
                        nop = mybir.InstNoOp(
                            name=f"antsplitw-{ctr}",
                            engine=inst.engine,
                            sync_info=mybir.SyncInfo(on_update=[], on_wait=[w]),
                        )
                        ctr += 1
                        new_list.append(nop)
                    si.on_wait = waits[-max_waits:]
                new_list.append(inst)
            bb.instructions[:] = new_list
    return ctr


def build_program(reps=1):
    nc = bass.Bass(num_devices=N_CORES)

    xT = nc.dram_tensor("xT", [D, N], BF16, kind="ExternalInput")
    wq = nc.dram_tensor("wq", [128, HL * KC * HD], BF16, kind="ExternalInput")
    wk = nc.dram_tensor("wk", [128, HL * KC * HD], BF16, kind="ExternalInput")
    wv = nc.dram_tensor("wv", [128, KC * 512], BF16, kind="ExternalInput")
    wo = nc.dram_tensor("wo", [128, H_TOT * 512], BF16, kind="ExternalInput")
    cc = nc.dram_tensor("cc", [128, N], BF16, kind="ExternalInput")
    ss = nc.dram_tensor("ss", [128, N], BF16, kind="ExternalInput")
    mask_in = nc.dram_tensor("mask", [128, 896], BF16, kind="ExternalInput")
    ones_col_in = nc.dram_tensor("ones_col", [128, 1], BF16, kind="ExternalInput")
    ones_row_in = nc.dram_tensor("ones_row", [1, 128], F32R, kind="ExternalInput")
    out = nc.dram_tensor("out", [N, 512], F32, kind="ExternalOutput")

    with nc.allow_low_precision(reason="bf16 matmul pipeline"):
        with tile.TileContext(nc) as tc:
            for rep in range(reps):
                _emit_rep(nc, tc, rep, xT, wq, wk, wv, wo, cc, ss, mask_in,
                          ones_col_in, ones_row_in, out)

    split_multi_waits(nc)
    return nc


def _emit_rep(nc, tc, rep, xT, wq, wk, wv, wo, cc, ss, mask_in,
              ones_col_in, ones_row_in, out):
    with ExitStack() as rep_ctx:
        const = rep_ctx.enter_context(tc.tile_pool(name=f"const{rep}", bufs=1))
        qk_pool = rep_ctx.enter_context(tc.tile_pool(name=f"qk{rep}", bufs=8))
        vn_pool = rep_ctx.enter_context(tc.tile_pool(name=f"vn{rep}", bufs=16))
        yn_pool = rep_ctx.enter_context(tc.tile_pool(name=f"yn{rep}", bufs=4))
        pt_pool = rep_ctx.enter_context(tc.tile_pool(name=f"pt{rep}", bufs=8))
        sm_pool = rep_ctx.enter_context(tc.tile_pool(name=f"sm{rep}", bufs=2))
        psS = rep_ctx.enter_context(
            tc.tile_pool(name=f"psS{rep}", bufs=2, space="PSUM"))
        psY = rep_ctx.enter_context(
            tc.tile_pool(name=f"psY{rep}", bufs=2, space="PSUM"))
        psD = rep_ctx.enter_context(
            tc.tile_pool(name=f"psD{rep}", bufs=1, space="PSUM"))
        psR = rep_ctx.enter_context(
            tc.tile_pool(name=f"psR{rep}", bufs=1, space="PSUM"))
        dram = rep_ctx.enter_context(
            tc.tile_pool(name=f"dram{rep}", bufs=1, space="DRAM"))

        mask_t = const.tile([128, 896], BF16, tag="mask")
        nc.gpsimd.dma_start(mask_t[:], mask_in[:])
        cc_t = const.tile([128, N], BF16, tag="cc")
        nc.gpsimd.dma_start(cc_t[:], cc[:])
        ss_t = const.tile([128, N], BF16, tag="ss")
        nc.gpsimd.dma_start(ss_t[:], ss[:])
        ones_col = const.tile([128, 1], BF16, tag="ones_col")
        nc.gpsimd.dma_start(ones_col[:], ones_col_in[:])
        ones_row = const.tile([1, 128], F32R, tag="ones_row")
        nc.gpsimd.dma_start(ones_row[:], ones_row_in[:])

        y_bounce = [dram.tile([HD, N], BF16, name=f"yb{rep}_{h}")
                    for h in range(HL)]
        y_gather = [dram.tile([4 * HD, N], BF16, name=f"yg{rep}_{h}")
                    for h in range(HL)]

        # persistent per-head q/k (RoPE'd, bf16) and v (natural layout)
        qr = [qk_pool.tile([128, N], BF16, tag="qr", name=f"qr{rep}_{h}")
              for h in range(HL)]
        kr = [qk_pool.tile([128, N], BF16, tag="kr", name=f"kr{rep}_{h}")
              for h in range(HL)]
        vn = [vn_pool.tile([128, 512], BF16, tag="vn", name=f"vn{rep}_{i}")
              for i in range(KC)]
        yn = [yn_pool.tile([128, 512], BF16, tag="yn", name=f"yn{rep}_{i}")
              for i in range(4 * NT)]

        def emit_a_qk(hl):
            for wdram, dst in ((wq, qr), (wk, kr)):
                wt = w_pool.tile([128, KC * HD], BF16, tag="wqk")
                nc.sync.dma_start(
                    wt[:], wdram[:, hl * KC * HD:(hl + 1) * KC * HD])
                for jn in range(NT):
                    acc = psA.tile([128, 512], F32, tag="psA",
                                   name=f"psA{rep}_{hl}_{jn}")
                    for k in range(KC):
                        nc.tensor.matmul(
                            acc[:], wt[:, k * HD:(k + 1) * HD],
                            xh[k][:, jn * 512:(jn + 1) * 512],
                            start=(k == 0), stop=(k == KC - 1))
                    bs = slice(jn * 512, (jn + 1) * 512)
                    q0 = rope_pool.tile([128, 512], BF16, tag="q0", bufs=2)
                    nc.vector.tensor_copy(q0[:], acc[:])
                    sw = rope_pool.tile([128, 512], BF16, tag="sw", bufs=2)
                    nc.sync.dma_start(sw[0:64, :], q0[64:128, :])
                    nc.sync.dma_start(sw[64:128, :], q0[0:64, :])
                    t1 = rope_pool.tile([128, 512], BF16, tag="t1", bufs=2)
                    nc.vector.tensor_mul(t1[:], q0[:], cc_t[:, bs])
                    nc.vector.tensor_mul(sw[:], sw[:], ss_t[:, bs])
                    nc.vector.tensor_add(dst[hl][:, bs], t1[:], sw[:])

        def emit_b(hl):
            for jn in range(NT):
                ims = list(range(min(16, 4 * jn + 4)))
                pts = []
                for im in ims:
                    s = psS.tile([128, 512], F32, tag="psS")
                    nc.tensor.matmul(
                        s[:], kr[hl][:, im * 128:(im + 1) * 128],
                        qr[hl][:, jn * 512:(jn + 1) * 512],
                        start=True, stop=True)
                    pt = pt_pool.tile([128, 512], BF16, tag="pt")
                    if im >= 4 * jn:
                        k2 = im - 4 * jn
                        pe = pt_pool.tile([128, 512], BF16, tag="pe", bufs=2)
                        nc.scalar.activation(pe[:], s[:], ACT_EXP,
                                             scale=INV_SQRT_HD)
                        nc.vector.tensor_mul(
                            pt[:], pe[:],
                            mask_t[:, (3 - k2) * 128:(3 - k2) * 128 + 512])
                    else:
                        nc.scalar.activation(pt[:], s[:], ACT_EXP,
                                             scale=INV_SQRT_HD)
                    pts.append(pt)
                y_acc = psY.tile([128, 512], F32, tag="psY")
                for idx, pt in enumerate(pts):
                    nc.tensor.matmul(
                        y_acc[:], vn[ims[idx]][:, hl * HD:(hl + 1) * HD],
                        pt[:], start=(idx == 0), stop=(idx == len(pts) - 1))
                den = psD.tile([1, 512], F32, tag="psD")
                for idx, pt in enumerate(pts):
                    nc.tensor.matmul(
                        den[:], ones_col[:], pt[:],
                        start=(idx == 0), stop=(idx == len(pts) - 1))
                den_inv = sm_pool.tile([1, 512], F32R, tag="den_inv")
                nc.scalar.activation(den_inv[:], den[:], ACT_LN)
                nc.scalar.activation(den_inv[:], den_inv[:], ACT_EXP,
                                     scale=-1.0)
                rep_ps = psR.tile([128, 512], F32, tag="psR")
                nc.tensor.matmul(rep_ps[:], ones_row[:], den_inv[:],
                                 start=True, stop=True)
                rinv = sm_pool.tile([128, 512], BF16, tag="rinv", bufs=2)
                nc.scalar.activation(rinv[:], rep_ps[:], ACT_COPY)
                ynt = yn[hl * NT + jn]
                nc.vector.tensor_mul(ynt[:], y_acc[:], rinv[:])
                nc.gpsimd.dma_start(
                    y_bounce[hl][:, jn * 512:(jn + 1) * 512], ynt[:])
            nc.gpsimd.collective_compute(
                "AllGather",
                mybir.AluOpType.bypass,
                replica_groups=[[0, 1, 2, 3], [4, 5, 6, 7]],
                ins=[y_bounce[hl].opt()],
                outs=[y_gather[hl].opt()],
            )

        with ExitStack() as ctx:
            xh_pool = ctx.enter_context(tc.tile_pool(name=f"xh{rep}", bufs=16))
            w_pool = ctx.enter_context(tc.tile_pool(name=f"w{rep}", bufs=2))
            wv_pool = ctx.enter_context(tc.tile_pool(name=f"wv{rep}", bufs=1))
            rope_pool = ctx.enter_context(
                tc.tile_pool(name=f"rope{rep}", bufs=6))
            psA = ctx.enter_context(
                tc.tile_pool(name=f"psA{rep}", bufs=2, space="PSUM"))

            # ---- loads ----------------------------------------------------
            wv_t = wv_pool.tile([128, KC * 512], BF16, tag="wv")
            nc.sync.dma_start(wv_t[:], wv[:])
            xh = []
            for k in range(KC):
                xt = xh_pool.tile([128, N], BF16, tag="xh")
                eng = nc.sync if k % 2 == 0 else nc.gpsimd
                eng.dma_start(xt[:], xT[k * 128:(k + 1) * 128, :])
                xh.append(xt)

            # ---- stage A-v: v projection (natural layout) -----------------
            for nchunk in range(16):
                vacc = psA.tile([128, 512], F32, tag="psA",
                                name=f"psAv{rep}_{nchunk}")
                for k in range(KC):
                    nc.tensor.matmul(
                        vacc[:],
                        xh[k][:, nchunk * 128:(nchunk + 1) * 128],
                        wv_t[:, k * 512:(k + 1) * 512],
                        start=(k == 0), stop=(k == KC - 1))
                nc.vector.tensor_copy(vn[nchunk][:], vacc[:])

            # ---- stage A-qk + RoPE interleaved with stage B per head ------
            for hl in range(HL):
                emit_a_qk(hl)
                if hl < HL - 1:
                    emit_b(hl)

        # ---- stage C pools open early so wo/yg loads overlap B(h3) --------
        with ExitStack() as ctx:
            wo_pool = ctx.enter_context(tc.tile_pool(name=f"wo{rep}", bufs=1))
            yg_pool = ctx.enter_context(tc.tile_pool(name=f"yg{rep}", bufs=16))
            od_pool = ctx.enter_context(tc.tile_pool(name=f"od{rep}", bufs=4))
            psC = ctx.enter_context(
                tc.tile_pool(name=f"psC{rep}", bufs=2, space="PSUM"))
            wo_t = wo_pool.tile([128, H_TOT * 512], BF16, tag="wo")
            nc.sync.dma_start(wo_t[:], wo[:])
            ygs = [None] * 16
            for hl in range(HL - 1):
                for r in range(4):
                    yg = yg_pool.tile([128, N], BF16, tag="yg")
                    eng = nc.sync if r % 2 == 0 else nc.gpsimd
                    eng.dma_start(
                        yg[:], y_gather[hl][r * 128:(r + 1) * 128, :])
                    ygs[hl * 4 + r] = yg

            emit_b(HL - 1)

            for r in range(4):
                yg = yg_pool.tile([128, N], BF16, tag="yg")
                eng = nc.sync if r % 2 == 0 else nc.gpsimd
                eng.dma_start(
                    yg[:], y_gather[HL - 1][r * 128:(r + 1) * 128, :])
                ygs[(HL - 1) * 4 + r] = yg

            # ---- stage C: o-projection, one output tile at a time ---------
            # chunk c2 = hl*4 + r maps to rows r*128 of y_gather[hl]
            for nt_ in range(16):
                acc = psC.tile([128, 512], F32, tag="psC",
                               name=f"psC{rep}_{nt_}")
                for c2 in range(16):
                    nc.tensor.matmul(
                        acc[:],
                        ygs[c2][:, nt_ * 128:(nt_ + 1) * 128],
                        wo_t[:, c2 * 512:(c2 + 1) * 512],
                        start=(c2 == 0), stop=(c2 == 15))
                od = od_pool.tile([128, 512], F32, tag="od")
                nc.vector.tensor_copy(od[:], acc[:])
                eng = nc.sync if nt_ % 2 == 0 else nc.gpsimd
                eng.dma_start(out[nt_ * 128:(nt_ + 1) * 128, :], od[:])


# ---------------------------------------------------------------------------
# host side
# ---------------------------------------------------------------------------

_DEINT = np.concatenate([np.arange(0, HD, 2), np.arange(1, HD, 2)])


def _bf16(a):
    return np.ascontiguousarray(a.astype(ml_dtypes.bfloat16))


def make_host_inputs(x, W_qkv, W_o):
    """Build the 8 per-core input dicts from the full problem inputs."""
    x = np.ascontiguousarray(np.asarray(x, dtype=np.float32))
    W_qkv = np.asarray(W_qkv, dtype=np.float32)
    W_o = np.asarray(W_o, dtype=np.float32)

    # RoPE tables, transposed + deinterleaved + duplicated/sign-folded
    theta = 1.0 / (ROPE_BASE ** (np.arange(0, HD, 2, dtype=np.float64) / HD))
    freqs = np.arange(N, dtype=np.float64)[:, None] * theta[None, :]  # [N, 64]
    cosT = np.cos(freqs).T.astype(np.float32)  # [64, N]
    sinT = np.sin(freqs).T.astype(np.float32)
    cc = np.concatenate([cosT, cosT], axis=0)  # [128, N]
    ss = np.concatenate([-sinT, sinT], axis=0)  # [128, N]

    # causal masks: M[i, t] = 1 if t >= i + 384 (shifted upper-tri family)
    i_idx = np.arange(128)[:, None]
    t_idx = np.arange(896)[None, :]
    mask = (t_idx >= i_idx + 384).astype(np.float32)

    ones_col = np.ones((128, 1), dtype=np.float32)
    ones_row = np.ones((1, 128), dtype=np.float32)

    # deinterleaved q/k weights: [D, H_TOT, HD]
    wq_full = W_qkv[:, 0 * D:1 * D].reshape(D, H_TOT, HD)[:, :, _DEINT]
    wk_full = W_qkv[:, 1 * D:2 * D].reshape(D, H_TOT, HD)[:, :, _DEINT]
    wv_full = W_qkv[:, 2 * D:3 * D]

    in_maps = []
    for c in range(N_CORES):
        b, g = divmod(c, 4)
        heads = slice(4 * g, 4 * g + 4)
        # wq/wk: [128, HL*KC*HD], block (hl*KC + k) is W[k*128:(k+1)*128,
        # head 4g+hl deint cols]
        wq_sel = wq_full[:, heads, :]  # [D, HL, HD]
        wk_sel = wk_full[:, heads, :]
        wq_pack = (wq_sel.reshape(KC, 128, HL, HD)
                   .transpose(1, 2, 0, 3).reshape(128, HL * KC * HD))
        wk_pack = (wk_sel.reshape(KC, 128, HL, HD)
                   .transpose(1, 2, 0, 3).reshape(128, HL * KC * HD))
        # wv: [128, KC*512], block k is Wv[k*128:(k+1)*128, 512g:512g+512]
        wv_pack = (wv_full[:, 512 * g:512 * g + 512]
                   .reshape(KC, 128, 512).transpose(1, 0, 2)
                   .reshape(128, KC * 512))
        # wo: [128, 16*512], chunk c2 = hl*4+r is W_o rows of global head
        # 4r+hl, columns 512g:512g+512
        wo_blocks = []
        for hl in range(HL):
            for r in range(4):
                gh = 4 * r + hl
                wo_blocks.append(
                    W_o[gh * 128:(gh + 1) * 128, 512 * g:512 * g + 512])
        wo_pack = (np.stack(wo_blocks, axis=0)  # [16, 128, 512]
                   .transpose(1, 0, 2).reshape(128, H_TOT * 512))
        in_maps.append({
            "xT": _bf16(x[b].T),
            "wq": _bf16(wq_pack),
            "wk": _bf16(wk_pack),
            "wv": _bf16(wv_pack),
            "wo": _bf16(wo_pack),
            "cc": _bf16(cc),
            "ss": _bf16(ss),
            "mask": _bf16(mask),
            "ones_col": _bf16(ones_col),
            "ones_row": ones_row,
        })
    return in_maps


def assemble_output(results):
    out = np.empty((B, N, D), dtype=np.float32)
    for c in range(N_CORES):
        b, g = divmod(c, 4)
        out[b, :, 512 * g:512 * g + 512] = results[c]["out"]
    return out


_PROGRAM = {}


def get_program(reps=1):
    if reps not in _PROGRAM:
        _PROGRAM[reps] = build_program(reps=reps)
    return _PROGRAM[reps]


def run(x, W_qkv, W_o, reps=1, **spmd_kwargs):
    nc = get_program(reps=reps)
    in_maps = make_host_inputs(x, W_qkv, W_o)
    res = run_bass_kernel_spmd(nc, in_maps, list(range(N_CORES)),
                               **spmd_kwargs)
    return assemble_output(res.results), res


def kernel(x, W_qkv, W_o):
    return run(x, W_qkv, W_o)[0]


if __name__ == "__main__":
    rng = np.random.default_rng(0)
    x = rng.standard_normal((B, N, D), dtype=np.float32)
    Wq = (rng.standard_normal((D, 3 * D), dtype=np.float32) * D ** -0.5)
    Wo = (rng.standard_normal((D, D), dtype=np.float32) * D ** -0.5)
    y = kernel(x, Wq, Wo)
    print("out:", y.shape, y.dtype, np.abs(y).max())


# revision 12
# speedup vs baseline: 1.1261x; 1.1261x over previous
"""Causal self-attention with RoPE for B=2, N=2048, D=2048, 16 heads,
distributed over 8 trn2 NeuronCores.

Sharding: core c = (b, g) with b = c // 4 (batch), g = c % 4 (head group of 4
heads).  Each core computes qkv projections + RoPE + causal attention for its
4 heads on its batch; per-head AllGathers ship each head's y^T to the other 3
cores of the batch while later heads still compute; each core then computes a
disjoint 512-column slice of the final o-projection.

v2 design notes (vs the fp32r two-half baseline):
- All matmul operands are bf16 (PSUM accumulation stays fp32).  bf16 enables
  FastWeightLoad (halves the per-matmul LDWEIGHTS cost that dominated the
  fp32r version), 2-4x DVE throughput, and half the DMA bytes.
- Stage A holds all 16 k-chunks of x^T in SBUF at once (bf16 makes it fit)
  and accumulates each qkv output tile across the full contraction in one
  PSUM bank: no half-pass SBUF adds, one PSUM->SBUF copy per tile, RoPE
  applied on the copy's output.
- Weights are host-packed so each logical stream is one large DMA.
- Softmax uses no max-subtraction (scores are O(1) by construction), masks
  multiply after exp, and row-normalization happens on y^T via a
  matmul-replicated reciprocal of the ones-matmul denominator.
"""

import numpy as np
from contextlib import ExitStack

import ml_dtypes

import concourse.bass as bass
import concourse.tile as tile
import concourse.mybir as mybir
from concourse.bass_utils import run_bass_kernel_spmd

F32 = mybir.dt.float32
F32R = mybir.dt.float32r
BF16 = mybir.dt.bfloat16

B = 2
N = 2048
D = 2048
H_TOT = 16
HD = 128  # head dim
HL = 4  # heads per core
N_CORES = 8
ROPE_BASE = 10000.0
INV_SQRT_HD = 1.0 / float(np.sqrt(HD))

NT = N // 512  # 4 n-tiles of 512
KC = D // 128  # 16 contraction chunks
ACT_COPY = mybir.ActivationFunctionType.Copy
ACT_EXP = mybir.ActivationFunctionType.Exp
ACT_LN = mybir.ActivationFunctionType.Ln


def split_multi_waits(nc, max_waits=1):
    """This container's walrus supports a single sync-wait per instruction;
    move extra waits onto preceding same-engine NoOps."""
    ctr = 0
    for f in nc.m.functions:
        for bb in f.blocks:
            new_list = []
            for inst in bb.instructions:
                si = inst.sync_info
                if si is not None and len(si.on_wait) > max_waits:
                    waits = list(si.on_wait)
                    for w in waits[:-max_waits]:
                        nop = mybir.InstNoOp(
                            name=f"antsplitw-{ctr}",
                            engine=inst.engine,
                            sync_info=mybir.SyncInfo(on_update=[], on_wait=[w]),
                        )
                        ctr += 1
                        new_list.append(nop)
                    si.on_wait = waits[-max_waits:]
                new_list.append(inst)
            bb.instructions[:] = new_list
    return ctr


def build_program(reps=1):
    nc = bass.Bass(num_devices=N_CORES)

    xT = nc.dram_tensor("xT", [D, N], BF16, kind="ExternalInput")
    wq = nc.dram_tensor("wq", [128, HL * KC * HD], BF16, kind="ExternalInput")
    wk = nc.dram_tensor("wk", [128, HL * KC * HD], BF16, kind="ExternalInput")
    wv = nc.dram_tensor("wv", [128, KC * 512], BF16, kind="ExternalInput")
    wo = nc.dram_tensor("wo", [128, H_TOT * 512], BF16, kind="ExternalInput")
    cc = nc.dram_tensor("cc", [128, N], BF16, kind="ExternalInput")
    ss = nc.dram_tensor("ss", [128, N], BF16, kind="ExternalInput")
    mask_in = nc.dram_tensor("mask", [128, 896], BF16, kind="ExternalInput")
    ones_col_in = nc.dram_tensor("ones_col", [128, 1], BF16, kind="ExternalInput")
    ones_row_in = nc.dram_tensor("ones_row", [1, 128], F32R, kind="ExternalInput")
    out = nc.dram_tensor("out", [N, 512], F32, kind="ExternalOutput")

    with nc.allow_low_precision(reason="bf16 matmul pipeline"):
        with tile.TileContext(nc) as tc:
            for rep in range(reps):
                _emit_rep(nc, tc, rep, xT, wq, wk, wv, wo, cc, ss, mask_in,
                          ones_col_in, ones_row_in, out)

    split_multi_waits(nc)
    return nc


def _emit_rep(nc, tc, rep, xT, wq, wk, wv, wo, cc, ss, mask_in,
              ones_col_in, ones_row_in, out):
    with ExitStack() as rep_ctx:
        const = rep_ctx.enter_context(tc.tile_pool(name=f"const{rep}", bufs=1))
        qk_pool = rep_ctx.enter_context(tc.tile_pool(name=f"qk{rep}", bufs=8))
        vn_pool = rep_ctx.enter_context(tc.tile_pool(name=f"vn{rep}", bufs=16))
        yn_pool = rep_ctx.enter_context(tc.tile_pool(name=f"yn{rep}", bufs=4))
        pt_pool = rep_ctx.enter_context(tc.tile_pool(name=f"pt{rep}", bufs=8))
        sm_pool = rep_ctx.enter_context(tc.tile_pool(name=f"sm{rep}", bufs=2))
        dram = rep_ctx.enter_context(
            tc.tile_pool(name=f"dram{rep}", bufs=1, space="DRAM"))

        mask_t = const.tile([128, 896], BF16, tag="mask")
        nc.gpsimd.dma_start(mask_t[:], mask_in[:])
        cc_t = const.tile([128, N], BF16, tag="cc")
        nc.gpsimd.dma_start(cc_t[:], cc[:])
        ss_t = const.tile([128, N], BF16, tag="ss")
        nc.gpsimd.dma_start(ss_t[:], ss[:])
        ones_col = const.tile([128, 1], BF16, tag="ones_col")
        nc.gpsimd.dma_start(ones_col[:], ones_col_in[:])
        ones_row = const.tile([1, 128], F32R, tag="ones_row")
        nc.gpsimd.dma_start(ones_row[:], ones_row_in[:])

        # head 3's bounce/gather is split into two column halves so its
        # AllGather can fire mid-B(3) and hide under stage C's first n-group
        y_bounce = [dram.tile([HD, N], BF16, name=f"yb{rep}_{h}")
                    for h in range(HL - 1)]
        y_gather = [dram.tile([4 * HD, N], BF16, name=f"yg{rep}_{h}")
                    for h in range(HL - 1)]
        yb3 = [dram.tile([HD, N // 2], BF16, name=f"yb3{rep}_{i}")
               for i in range(2)]
        yg3 = [dram.tile([4 * HD, N // 2], BF16, name=f"yg3{rep}_{i}")
               for i in range(2)]

        # persistent per-head q/k (RoPE'd, bf16) and v (natural layout)
        qr = [qk_pool.tile([128, N], BF16, tag="qr", name=f"qr{rep}_{h}")
              for h in range(HL)]
        kr = [qk_pool.tile([128, N], BF16, tag="kr", name=f"kr{rep}_{h}")
              for h in range(HL)]
        vn = [vn_pool.tile([128, 512], BF16, tag="vn", name=f"vn{rep}_{i}")
              for i in range(KC)]
        yn = [yn_pool.tile([128, 512], BF16, tag="yn", name=f"yn{rep}_{i}")
              for i in range(4 * NT)]

        def emit_ag(ins_t, outs_t):
            nc.gpsimd.collective_compute(
                "AllGather",
                mybir.AluOpType.bypass,
                replica_groups=[[0, 1, 2, 3], [4, 5, 6, 7]],
                ins=[ins_t.opt()],
                outs=[outs_t.opt()],
            )

        def emit_a_qk(hl):
            for wdram, dst in ((wq, qr), (wk, kr)):
                wt = w_pool.tile([128, KC * HD], BF16, tag="wqk")
                nc.sync.dma_start(
                    wt[:], wdram[:, hl * KC * HD:(hl + 1) * KC * HD])
                for jn in range(NT):
                    acc = psA.tile([128, 512], F32, tag="psA",
                                   name=f"psA{rep}_{hl}_{jn}")
                    for k in range(KC):
                        nc.tensor.matmul(
                            acc[:], wt[:, k * HD:(k + 1) * HD],
                            xh[k][:, jn * 512:(jn + 1) * 512],
                            start=(k == 0), stop=(k == KC - 1))
                    bs = slice(jn * 512, (jn + 1) * 512)
                    q0 = rope_pool.tile([128, 512], BF16, tag="q0", bufs=2)
                    nc.vector.tensor_copy(q0[:], acc[:])
                    sw = rope_pool.tile([128, 512], BF16, tag="sw", bufs=2)
                    nc.sync.dma_start(sw[0:64, :], q0[64:128, :])
                    nc.sync.dma_start(sw[64:128, :], q0[0:64, :])
                    t1 = rope_pool.tile([128, 512], BF16, tag="t1", bufs=2)
                    nc.vector.tensor_mul(t1[:], q0[:], cc_t[:, bs])
                    nc.vector.tensor_mul(sw[:], sw[:], ss_t[:, bs])
                    nc.vector.tensor_add(dst[hl][:, bs], t1[:], sw[:])

        def emit_b(hl):
            for jn in range(NT):
                ims = list(range(min(16, 4 * jn + 4)))
                pts = []
                for im in ims:
                    s = psS.tile([128, 512], F32, tag="psS")
                    nc.tensor.matmul(
                        s[:], kr[hl][:, im * 128:(im + 1) * 128],
                        qr[hl][:, jn * 512:(jn + 1) * 512],
                        start=True, stop=True)
                    pt = pt_pool.tile([128, 512], BF16, tag="pt")
                    if im >= 4 * jn:
                        k2 = im - 4 * jn
                        pe = pt_pool.tile([128, 512], BF16, tag="pe", bufs=2)
                        nc.scalar.activation(pe[:], s[:], ACT_EXP,
                                             scale=INV_SQRT_HD)
                        nc.vector.tensor_mul(
                            pt[:], pe[:],
                            mask_t[:, (3 - k2) * 128:(3 - k2) * 128 + 512])
                    else:
                        nc.scalar.activation(pt[:], s[:], ACT_EXP,
                                             scale=INV_SQRT_HD)
                    pts.append(pt)
                y_acc = psY.tile([128, 512], F32, tag="psY")
                for idx, pt in enumerate(pts):
                    nc.tensor.matmul(
                        y_acc[:], vn[ims[idx]][:, hl * HD:(hl + 1) * HD],
                        pt[:], start=(idx == 0), stop=(idx == len(pts) - 1))
                den = psD.tile([1, 512], F32, tag="psD")
                for idx, pt in enumerate(pts):
                    nc.tensor.matmul(
                        den[:], ones_col[:], pt[:],
                        start=(idx == 0), stop=(idx == len(pts) - 1))
                den_inv = sm_pool.tile([1, 512], F32R, tag="den_inv")
                nc.scalar.activation(den_inv[:], den[:], ACT_LN)
                nc.scalar.activation(den_inv[:], den_inv[:], ACT_EXP,
                                     scale=-1.0)
                rep_ps = psR.tile([128, 512], F32, tag="psR")
                nc.tensor.matmul(rep_ps[:], ones_row[:], den_inv[:],
                                 start=True, stop=True)
                rinv = sm_pool.tile([128, 512], BF16, tag="rinv", bufs=2)
                nc.scalar.activation(rinv[:], rep_ps[:], ACT_COPY)
                ynt = yn[hl * NT + jn]
                nc.vector.tensor_mul(ynt[:], y_acc[:], rinv[:])
                if hl < HL - 1:
                    nc.gpsimd.dma_start(
                        y_bounce[hl][:, jn * 512:(jn + 1) * 512], ynt[:])
                else:
                    nc.gpsimd.dma_start(
                        yb3[jn // 2][:, (jn % 2) * 512:(jn % 2) * 512 + 512],
                        ynt[:])
                    if jn == 1:
                        emit_ag(yb3[0], yg3[0])
                    elif jn == 3:
                        emit_ag(yb3[1], yg3[1])
            if hl < HL - 1:
                emit_ag(y_bounce[hl], y_gather[hl])

        with ExitStack() as ctxB:
            psS = ctxB.enter_context(
                tc.tile_pool(name=f"psS{rep}", bufs=2, space="PSUM"))
            psY = ctxB.enter_context(
                tc.tile_pool(name=f"psY{rep}", bufs=2, space="PSUM"))
            psD = ctxB.enter_context(
                tc.tile_pool(name=f"psD{rep}", bufs=1, space="PSUM"))
            psR = ctxB.enter_context(
                tc.tile_pool(name=f"psR{rep}", bufs=1, space="PSUM"))

            with ExitStack() as ctx:
                xh_pool = ctx.enter_context(
                    tc.tile_pool(name=f"xh{rep}", bufs=16))
                w_pool = ctx.enter_context(tc.tile_pool(name=f"w{rep}", bufs=2))
                wv_pool = ctx.enter_context(
                    tc.tile_pool(name=f"wv{rep}", bufs=1))
                rope_pool = ctx.enter_context(
                    tc.tile_pool(name=f"rope{rep}", bufs=6))
                psA = ctx.enter_context(
                    tc.tile_pool(name=f"psA{rep}", bufs=2, space="PSUM"))

                # ---- loads ------------------------------------------------
                wv_t = wv_pool.tile([128, KC * 512], BF16, tag="wv")
                nc.sync.dma_start(wv_t[:], wv[:])
                xh = []
                for k in range(KC):
                    xt = xh_pool.tile([128, N], BF16, tag="xh")
                    nc.sync.dma_start(xt[:], xT[k * 128:(k + 1) * 128, :])
                    xh.append(xt)

                # ---- stage A-v: v projection (natural layout) -------------
                for nchunk in range(16):
                    vacc = psA.tile([128, 512], F32, tag="psA",
                                    name=f"psAv{rep}_{nchunk}")
                    for k in range(KC):
                        nc.tensor.matmul(
                            vacc[:],
                            xh[k][:, nchunk * 128:(nchunk + 1) * 128],
                            wv_t[:, k * 512:(k + 1) * 512],
                            start=(k == 0), stop=(k == KC - 1))
                    nc.vector.tensor_copy(vn[nchunk][:], vacc[:])

                # ---- stage A-qk + RoPE interleaved with stage B per head --
                for hl in range(HL):
                    emit_a_qk(hl)
                    if hl < HL - 1:
                        emit_b(hl)

            # ctx closed: xh/w/wv/rope SBUF freed -> stage C loads overlap
            # B(3).  These pools live until rep_ctx exits.
            wo_pool = rep_ctx.enter_context(
                tc.tile_pool(name=f"wo{rep}", bufs=1))
            yg_pool = rep_ctx.enter_context(
                tc.tile_pool(name=f"yg{rep}", bufs=12))
            od_pool = rep_ctx.enter_context(
                tc.tile_pool(name=f"od{rep}", bufs=4))
            wo_t = wo_pool.tile([128, H_TOT * 512], BF16, tag="wo")
            nc.sync.dma_start(wo_t[:], wo[:])
            ygs = [None] * 12
            for hl in range(HL - 1):
                for r in range(4):
                    yg = yg_pool.tile([128, N], BF16, tag="yg")
                    nc.sync.dma_start(
                        yg[:], y_gather[hl][r * 128:(r + 1) * 128, :])
                    ygs[hl * 4 + r] = yg

            emit_b(HL - 1)

        # ctxB closed: all 6 B-phase PSUM banks free for stage C
        psC = rep_ctx.enter_context(
            tc.tile_pool(name=f"psC{rep}", bufs=8, space="PSUM"))
        yg3s = [[None] * 4, [None] * 4]
        for half in range(2):
            for r in range(4):
                yg = yg_pool.tile([128, N // 2], BF16, tag="yg3", bufs=8)
                eng = nc.sync if r % 2 == 0 else nc.gpsimd
                eng.dma_start(
                    yg[:], yg3[half][r * 128:(r + 1) * 128, :])
                yg3s[half][r] = yg

        # ---- stage C: o-projection, chunk-outer so head-3 chunks are last -
        # chunk c2 = hl*4 + r maps to rows r*128 of y_gather[hl]
        for ngrp in range(2):
            accs = [psC.tile([128, 512], F32, tag="psC",
                             name=f"psC{rep}_{ngrp}_{i}")
                    for i in range(8)]
            for c2 in range(16):
                hl, r = divmod(c2, 4)
                for t in range(8):
                    nt_ = ngrp * 8 + t
                    if hl < HL - 1:
                        lhsT = ygs[c2][:, nt_ * 128:(nt_ + 1) * 128]
                    else:
                        lhsT = yg3s[ngrp][r][:, t * 128:(t + 1) * 128]
                    nc.tensor.matmul(
                        accs[t][:], lhsT,
                        wo_t[:, c2 * 512:(c2 + 1) * 512],
                        start=(c2 == 0), stop=(c2 == 15))
            for t in range(8):
                od = od_pool.tile([128, 512], F32, tag="od")
                nc.vector.tensor_copy(od[:], accs[t][:])
                nt_ = ngrp * 8 + t
                eng = nc.sync if t % 2 == 0 else nc.gpsimd
                eng.dma_start(out[nt_ * 128:(nt_ + 1) * 128, :], od[:])


# ---------------------------------------------------------------------------
# host side
# ---------------------------------------------------------------------------

_DEINT = np.concatenate([np.arange(0, HD, 2), np.arange(1, HD, 2)])


def _bf16(a):
    return np.ascontiguousarray(a.astype(ml_dtypes.bfloat16))


def make_host_inputs(x, W_qkv, W_o):
    """Build the 8 per-core input dicts from the full problem inputs."""
    x = np.ascontiguousarray(np.asarray(x, dtype=np.float32))
    W_qkv = np.asarray(W_qkv, dtype=np.float32)
    W_o = np.asarray(W_o, dtype=np.float32)

    # RoPE tables, transposed + deinterleaved + duplicated/sign-folded
    theta = 1.0 / (ROPE_BASE ** (np.arange(0, HD, 2, dtype=np.float64) / HD))
    freqs = np.arange(N, dtype=np.float64)[:, None] * theta[None, :]  # [N, 64]
    cosT = np.cos(freqs).T.astype(np.float32)  # [64, N]
    sinT = np.sin(freqs).T.astype(np.float32)
    cc = np.concatenate([cosT, cosT], axis=0)  # [128, N]
    ss = np.concatenate([-sinT, sinT], axis=0)  # [128, N]

    # causal masks: M[i, t] = 1 if t >= i + 384 (shifted upper-tri family)
    i_idx = np.arange(128)[:, None]
    t_idx = np.arange(896)[None, :]
    mask = (t_idx >= i_idx + 384).astype(np.float32)

    ones_col = np.ones((128, 1), dtype=np.float32)
    ones_row = np.ones((1, 128), dtype=np.float32)

    # deinterleaved q/k weights: [D, H_TOT, HD]
    wq_full = W_qkv[:, 0 * D:1 * D].reshape(D, H_TOT, HD)[:, :, _DEINT]
    wk_full = W_qkv[:, 1 * D:2 * D].reshape(D, H_TOT, HD)[:, :, _DEINT]
    wv_full = W_qkv[:, 2 * D:3 * D]

    in_maps = []
    for c in range(N_CORES):
        b, g = divmod(c, 4)
        heads = slice(4 * g, 4 * g + 4)
        # wq/wk: [128, HL*KC*HD], block (hl*KC + k) is W[k*128:(k+1)*128,
        # head 4g+hl deint cols]
        wq_sel = wq_full[:, heads, :]  # [D, HL, HD]
        wk_sel = wk_full[:, heads, :]
        wq_pack = (wq_sel.reshape(KC, 128, HL, HD)
                   .transpose(1, 2, 0, 3).reshape(128, HL * KC * HD))
        wk_pack = (wk_sel.reshape(KC, 128, HL, HD)
                   .transpose(1, 2, 0, 3).reshape(128, HL * KC * HD))
        # wv: [128, KC*512], block k is Wv[k*128:(k+1)*128, 512g:512g+512]
        wv_pack = (wv_full[:, 512 * g:512 * g + 512]
                   .reshape(KC, 128, 512).transpose(1, 0, 2)
                   .reshape(128, KC * 512))
        # wo: [128, 16*512], chunk c2 = hl*4+r is W_o rows of global head
        # 4r+hl, columns 512g:512g+512
        wo_blocks = []
        for hl in range(HL):
            for r in range(4):
                gh = 4 * r + hl
                wo_blocks.append(
                    W_o[gh * 128:(gh + 1) * 128, 512 * g:512 * g + 512])
        wo_pack = (np.stack(wo_blocks, axis=0)  # [16, 128, 512]
                   .transpose(1, 0, 2).reshape(128, H_TOT * 512))
        in_maps.append({
            "xT": _bf16(x[b].T),
            "wq": _bf16(wq_pack),
            "wk": _bf16(wk_pack),
            "wv": _bf16(wv_pack),
            "wo": _bf16(wo_pack),
            "cc": _bf16(cc),
            "ss": _bf16(ss),
            "mask": _bf16(mask),
            "ones_col": _bf16(ones_col),
            "ones_row": ones_row,
        })
    return in_maps


def assemble_output(results):
    out = np.empty((B, N, D), dtype=np.float32)
    for c in range(N_CORES):
        b, g = divmod(c, 4)
        out[b, :, 512 * g:512 * g + 512] = results[c]["out"]
    return out


_PROGRAM = {}


def get_program(reps=1):
    if reps not in _PROGRAM:
        _PROGRAM[reps] = build_program(reps=reps)
    return _PROGRAM[reps]


def run(x, W_qkv, W_o, reps=1, **spmd_kwargs):
    nc = get_program(reps=reps)
    in_maps = make_host_inputs(x, W_qkv, W_o)
    res = run_bass_kernel_spmd(nc, in_maps, list(range(N_CORES)),
                               **spmd_kwargs)
    return assemble_output(res.results), res


def kernel(x, W_qkv, W_o):
    return run(x, W_qkv, W_o)[0]


if __name__ == "__main__":
    rng = np.random.default_rng(0)
    x = rng.standard_normal((B, N, D), dtype=np.float32)
    Wq = (rng.standard_normal((D, 3 * D), dtype=np.float32) * D ** -0.5)
    Wo = (rng.standard_normal((D, D), dtype=np.float32) * D ** -0.5)
    y = kernel(x, Wq, Wo)
    print("out:", y.shape, y.dtype, np.abs(y).max())


# revision 13
# speedup vs baseline: 1.1825x; 1.0500x over previous
"""Causal self-attention with RoPE for B=2, N=2048, D=2048, 16 heads,
distributed over 8 trn2 NeuronCores.

Sharding: core c = (b, g) with b = c // 4 (batch), g = c % 4 (head group of 4
heads).  Each core computes qkv projections + RoPE + causal attention for its
4 heads on its batch; per-head AllGathers ship each head's y^T to the other 3
cores of the batch while later heads still compute; each core then computes a
disjoint 512-column slice of the final o-projection.

v2 design notes (vs the fp32r two-half baseline):
- All matmul operands are bf16 (PSUM accumulation stays fp32).  bf16 enables
  FastWeightLoad (halves the per-matmul LDWEIGHTS cost that dominated the
  fp32r version), 2-4x DVE throughput, and half the DMA bytes.
- Stage A holds all 16 k-chunks of x^T in SBUF at once (bf16 makes it fit)
  and accumulates each qkv output tile across the full contraction in one
  PSUM bank: no half-pass SBUF adds, one PSUM->SBUF copy per tile, RoPE
  applied on the copy's output.
- Weights are host-packed so each logical stream is one large DMA.
- Softmax uses no max-subtraction (scores are O(1) by construction), masks
  multiply after exp, and row-normalization happens on y^T via a
  matmul-replicated reciprocal of the ones-matmul denominator.
"""

import numpy as np
from contextlib import ExitStack

import ml_dtypes

import concourse.bass as bass
import concourse.tile as tile
import concourse.mybir as mybir
from concourse.bass_utils import run_bass_kernel_spmd

F32 = mybir.dt.float32
F32R = mybir.dt.float32r
BF16 = mybir.dt.bfloat16

B = 2
N = 2048
D = 2048
H_TOT = 16
HD = 128  # head dim
HL = 4  # heads per core
N_CORES = 8
ROPE_BASE = 10000.0
INV_SQRT_HD = 1.0 / float(np.sqrt(HD))

NT = N // 512  # 4 n-tiles of 512
KC = D // 128  # 16 contraction chunks
ACT_COPY = mybir.ActivationFunctionType.Copy
ACT_EXP = mybir.ActivationFunctionType.Exp
ACT_LN = mybir.ActivationFunctionType.Ln


def split_multi_waits(nc, max_waits=1):
    """This container's walrus supports a single sync-wait per instruction;
    move extra waits onto preceding same-engine NoOps."""
    ctr = 0
    for f in nc.m.functions:
        for bb in f.blocks:
            new_list = []
            for inst in bb.instructions:
                si = inst.sync_info
                if si is not None and len(si.on_wait) > max_waits:
                    waits = list(si.on_wait)
                    for w in waits[:-max_waits]:
                        nop = mybir.InstNoOp(
                            name=f"antsplitw-{ctr}",
                            engine=inst.engine,
                            sync_info=mybir.SyncInfo(on_update=[], on_wait=[w]),
                        )
                        ctr += 1
                        new_list.append(nop)
                    si.on_wait = waits[-max_waits:]
                new_list.append(inst)
            bb.instructions[:] = new_list
    return ctr


def build_program(reps=1):
    nc = bass.Bass(num_devices=N_CORES)

    xT = nc.dram_tensor("xT", [D, N], BF16, kind="ExternalInput")
    wq = nc.dram_tensor("wq", [128, HL * KC * HD], BF16, kind="ExternalInput")
    wk = nc.dram_tensor("wk", [128, HL * KC * HD], BF16, kind="ExternalInput")
    wv = nc.dram_tensor("wv", [128, KC * 512], BF16, kind="ExternalInput")
    wo = nc.dram_tensor("wo", [128, H_TOT * 512], BF16, kind="ExternalInput")
    cc = nc.dram_tensor("cc", [128, N], BF16, kind="ExternalInput")
    ss = nc.dram_tensor("ss", [128, N], BF16, kind="ExternalInput")
    mask_in = nc.dram_tensor("mask", [128, 896], BF16, kind="ExternalInput")
    ones_col_in = nc.dram_tensor("ones_col", [128, 1], BF16, kind="ExternalInput")
    ones_row_in = nc.dram_tensor("ones_row", [1, 128], F32R, kind="ExternalInput")
    out = nc.dram_tensor("out", [N, 512], F32, kind="ExternalOutput")

    with nc.allow_low_precision(reason="bf16 matmul pipeline"):
        with tile.TileContext(nc) as tc:
            for rep in range(reps):
                _emit_rep(nc, tc, rep, xT, wq, wk, wv, wo, cc, ss, mask_in,
                          ones_col_in, ones_row_in, out)

    split_multi_waits(nc)
    return nc


def _emit_rep(nc, tc, rep, xT, wq, wk, wv, wo, cc, ss, mask_in,
              ones_col_in, ones_row_in, out):
    with ExitStack() as rep_ctx:
        const = rep_ctx.enter_context(tc.tile_pool(name=f"const{rep}", bufs=1))
        qk_pool = rep_ctx.enter_context(tc.tile_pool(name=f"qk{rep}", bufs=8))
        vn_pool = rep_ctx.enter_context(tc.tile_pool(name=f"vn{rep}", bufs=16))
        yn_pool = rep_ctx.enter_context(tc.tile_pool(name=f"yn{rep}", bufs=4))
        pt_pool = rep_ctx.enter_context(tc.tile_pool(name=f"pt{rep}", bufs=8))
        sm_pool = rep_ctx.enter_context(tc.tile_pool(name=f"sm{rep}", bufs=2))
        dram = rep_ctx.enter_context(
            tc.tile_pool(name=f"dram{rep}", bufs=1, space="DRAM"))

        mask_t = const.tile([128, 896], BF16, tag="mask")
        nc.gpsimd.dma_start(mask_t[:], mask_in[:])
        cc_t = const.tile([128, N], BF16, tag="cc")
        nc.gpsimd.dma_start(cc_t[:], cc[:])
        ss_t = const.tile([128, N], BF16, tag="ss")
        nc.gpsimd.dma_start(ss_t[:], ss[:])
        ones_col = const.tile([128, 1], BF16, tag="ones_col")
        nc.gpsimd.dma_start(ones_col[:], ones_col_in[:])
        ones_row = const.tile([1, 128], F32R, tag="ones_row")
        nc.gpsimd.dma_start(ones_row[:], ones_row_in[:])

        # head 3's bounce/gather is split into two column halves so its
        # AllGather can fire mid-B(3) and hide under stage C's first n-group
        y_bounce = [dram.tile([HD, N], BF16, name=f"yb{rep}_{h}")
                    for h in range(HL - 1)]
        y_gather = [dram.tile([4 * HD, N], BF16, name=f"yg{rep}_{h}")
                    for h in range(HL - 1)]
        yb3 = [dram.tile([HD, 512], BF16, name=f"yb3{rep}_{i}")
               for i in range(4)]
        yg3 = [dram.tile([4 * HD, 512], BF16, name=f"yg3{rep}_{i}")
               for i in range(4)]

        # persistent per-head q/k (RoPE'd, bf16) and v (natural layout)
        qr = [qk_pool.tile([128, N], BF16, tag="qr", name=f"qr{rep}_{h}")
              for h in range(HL)]
        kr = [qk_pool.tile([128, N], BF16, tag="kr", name=f"kr{rep}_{h}")
              for h in range(HL)]
        vn = [vn_pool.tile([128, 512], BF16, tag="vn", name=f"vn{rep}_{i}")
              for i in range(KC)]
        yn = [yn_pool.tile([128, 512], BF16, tag="yn", name=f"yn{rep}_{i}")
              for i in range(4 * NT)]

        def emit_ag(ins_t, outs_t):
            nc.gpsimd.collective_compute(
                "AllGather",
                mybir.AluOpType.bypass,
                replica_groups=[[0, 1, 2, 3], [4, 5, 6, 7]],
                ins=[ins_t.opt()],
                outs=[outs_t.opt()],
            )

        def emit_a_qk(hl):
            for wdram, dst in ((wq, qr), (wk, kr)):
                wt = w_pool.tile([128, KC * HD], BF16, tag="wqk")
                nc.sync.dma_start(
                    wt[:], wdram[:, hl * KC * HD:(hl + 1) * KC * HD])
                for jn in range(NT):
                    acc = psA.tile([128, 512], F32, tag="psA",
                                   name=f"psA{rep}_{hl}_{jn}")
                    for k in range(KC):
                        nc.tensor.matmul(
                            acc[:], wt[:, k * HD:(k + 1) * HD],
                            xh[k][:, jn * 512:(jn + 1) * 512],
                            start=(k == 0), stop=(k == KC - 1))
                    bs = slice(jn * 512, (jn + 1) * 512)
                    q0 = rope_pool.tile([128, 512], BF16, tag="q0", bufs=2)
                    nc.vector.tensor_copy(q0[:], acc[:])
                    sw = rope_pool.tile([128, 512], BF16, tag="sw", bufs=2)
                    nc.sync.dma_start(sw[0:64, :], q0[64:128, :])
                    nc.sync.dma_start(sw[64:128, :], q0[0:64, :])
                    t1 = rope_pool.tile([128, 512], BF16, tag="t1", bufs=2)
                    nc.vector.tensor_mul(t1[:], q0[:], cc_t[:, bs])
                    nc.vector.tensor_mul(sw[:], sw[:], ss_t[:, bs])
                    nc.vector.tensor_add(dst[hl][:, bs], t1[:], sw[:])

        def emit_b(hl):
            for jn in range(NT):
                ims = list(range(min(16, 4 * jn + 4)))
                pts = []
                for im in ims:
                    s = psS.tile([128, 512], F32, tag="psS")
                    nc.tensor.matmul(
                        s[:], kr[hl][:, im * 128:(im + 1) * 128],
                        qr[hl][:, jn * 512:(jn + 1) * 512],
                        start=True, stop=True)
                    pt = pt_pool.tile([128, 512], BF16, tag="pt")
                    if im >= 4 * jn:
                        k2 = im - 4 * jn
                        pe = pt_pool.tile([128, 512], BF16, tag="pe", bufs=2)
                        nc.scalar.activation(pe[:], s[:], ACT_EXP,
                                             scale=INV_SQRT_HD)
                        nc.vector.tensor_mul(
                            pt[:], pe[:],
                            mask_t[:, (3 - k2) * 128:(3 - k2) * 128 + 512])
                    else:
                        nc.scalar.activation(pt[:], s[:], ACT_EXP,
                                             scale=INV_SQRT_HD)
                    pts.append(pt)
                y_acc = psY.tile([128, 512], F32, tag="psY")
                for idx, pt in enumerate(pts):
                    nc.tensor.matmul(
                        y_acc[:], vn[ims[idx]][:, hl * HD:(hl + 1) * HD],
                        pt[:], start=(idx == 0), stop=(idx == len(pts) - 1))
                dsum = []
                for i in range(0, len(pts) - 1, 2):
                    ds = pt_pool.tile([128, 512], BF16, tag="ds", bufs=2)
                    nc.vector.tensor_add(ds[:], pts[i][:], pts[i + 1][:])
                    dsum.append(ds)
                if len(pts) % 2:
                    dsum.append(pts[-1])
                den = psD.tile([1, 512], F32, tag="psD")
                for idx, ds in enumerate(dsum):
                    nc.tensor.matmul(
                        den[:], ones_col[:], ds[:],
                        start=(idx == 0), stop=(idx == len(dsum) - 1))
                den_inv = sm_pool.tile([1, 512], F32R, tag="den_inv")
                nc.scalar.activation(den_inv[:], den[:], ACT_LN)
                nc.scalar.activation(den_inv[:], den_inv[:], ACT_EXP,
                                     scale=-1.0)
                rep_ps = psR.tile([128, 512], F32, tag="psR")
                nc.tensor.matmul(rep_ps[:], ones_row[:], den_inv[:],
                                 start=True, stop=True)
                rinv = sm_pool.tile([128, 512], BF16, tag="rinv", bufs=2)
                nc.scalar.activation(rinv[:], rep_ps[:], ACT_COPY)
                ynt = yn[hl * NT + jn]
                nc.vector.tensor_mul(ynt[:], y_acc[:], rinv[:])
                if hl < HL - 1:
                    nc.gpsimd.dma_start(
                        y_bounce[hl][:, jn * 512:(jn + 1) * 512], ynt[:])
                else:
                    nc.gpsimd.dma_start(yb3[jn][:], ynt[:])
                    emit_ag(yb3[jn], yg3[jn])
            if hl < HL - 1:
                emit_ag(y_bounce[hl], y_gather[hl])

        with ExitStack() as ctxB:
            psS = ctxB.enter_context(
                tc.tile_pool(name=f"psS{rep}", bufs=2, space="PSUM"))
            psY = ctxB.enter_context(
                tc.tile_pool(name=f"psY{rep}", bufs=2, space="PSUM"))
            psD = ctxB.enter_context(
                tc.tile_pool(name=f"psD{rep}", bufs=1, space="PSUM"))
            psR = ctxB.enter_context(
                tc.tile_pool(name=f"psR{rep}", bufs=1, space="PSUM"))

            with ExitStack() as ctx:
                xh_pool = ctx.enter_context(
                    tc.tile_pool(name=f"xh{rep}", bufs=16))
                w_pool = ctx.enter_context(tc.tile_pool(name=f"w{rep}", bufs=2))
                wv_pool = ctx.enter_context(
                    tc.tile_pool(name=f"wv{rep}", bufs=1))
                rope_pool = ctx.enter_context(
                    tc.tile_pool(name=f"rope{rep}", bufs=6))
                psA = ctx.enter_context(
                    tc.tile_pool(name=f"psA{rep}", bufs=2, space="PSUM"))

                # ---- loads ------------------------------------------------
                wv_t = wv_pool.tile([128, KC * 512], BF16, tag="wv")
                nc.sync.dma_start(wv_t[:], wv[:])
                xh = []
                for k in range(KC):
                    xt = xh_pool.tile([128, N], BF16, tag="xh")
                    eng = nc.sync if k % 2 == 0 else nc.scalar
                    eng.dma_start(xt[:], xT[k * 128:(k + 1) * 128, :])
                    xh.append(xt)

                # ---- stage A-v: v projection (natural layout) -------------
                for nchunk in range(16):
                    vacc = psA.tile([128, 512], F32, tag="psA",
                                    name=f"psAv{rep}_{nchunk}")
                    for k in range(KC):
                        nc.tensor.matmul(
                            vacc[:],
                            xh[k][:, nchunk * 128:(nchunk + 1) * 128],
                            wv_t[:, k * 512:(k + 1) * 512],
                            start=(k == 0), stop=(k == KC - 1))
                    nc.vector.tensor_copy(vn[nchunk][:], vacc[:])

                # ---- stage A-qk + RoPE interleaved with stage B per head --
                for hl in range(HL):
                    emit_a_qk(hl)
                    if hl < HL - 1:
                        emit_b(hl)

            # ctx closed: xh/w/wv/rope SBUF freed -> stage C loads overlap
            # B(3).  These pools live until rep_ctx exits.
            wo_pool = rep_ctx.enter_context(
                tc.tile_pool(name=f"wo{rep}", bufs=1))
            yg_pool = rep_ctx.enter_context(
                tc.tile_pool(name=f"yg{rep}", bufs=12))
            od_pool = rep_ctx.enter_context(
                tc.tile_pool(name=f"od{rep}", bufs=4))
            wo_t = wo_pool.tile([128, H_TOT * 512], BF16, tag="wo")
            nc.sync.dma_start(wo_t[:], wo[:])
            ygs = [None] * 12
            for hl in range(HL - 1):
                for r in range(4):
                    yg = yg_pool.tile([128, N], BF16, tag="yg")
                    nc.sync.dma_start(
                        yg[:], y_gather[hl][r * 128:(r + 1) * 128, :])
                    ygs[hl * 4 + r] = yg

            emit_b(HL - 1)

        # ctxB closed: all 6 B-phase PSUM banks free for stage C
        psC = rep_ctx.enter_context(
            tc.tile_pool(name=f"psC{rep}", bufs=8, space="PSUM"))
        yg3s = [[None] * 4 for _ in range(4)]
        for q in range(4):
            for r in range(4):
                yg = yg_pool.tile([128, 512], BF16, tag="yg3", bufs=16)
                eng = nc.sync if r % 2 == 0 else nc.gpsimd
                eng.dma_start(
                    yg[:], yg3[q][r * 128:(r + 1) * 128, :])
                yg3s[q][r] = yg

        # ---- stage C: o-projection, chunk-outer so head-3 chunks are last -
        # chunk c2 = hl*4 + r maps to rows r*128 of y_gather[hl]
        for ngrp in range(2):
            accs = [psC.tile([128, 512], F32, tag="psC",
                             name=f"psC{rep}_{ngrp}_{i}")
                    for i in range(8)]
            c2_order = (list(range(16)) if ngrp == 0
                        else list(range(12, 16)) + list(range(12)))
            for ci, c2 in enumerate(c2_order):
                hl, r = divmod(c2, 4)
                for t in range(8):
                    nt_ = ngrp * 8 + t
                    if hl < HL - 1:
                        lhsT = ygs[c2][:, nt_ * 128:(nt_ + 1) * 128]
                    else:
                        lhsT = yg3s[nt_ // 4][r][:, (nt_ % 4) * 128:
                                                 (nt_ % 4) * 128 + 128]
                    nc.tensor.matmul(
                        accs[t][:], lhsT,
                        wo_t[:, c2 * 512:(c2 + 1) * 512],
                        start=(ci == 0), stop=(ci == 15))
            for t in range(8):
                od = od_pool.tile([128, 512], F32, tag="od")
                if t % 2 == 0:
                    nc.vector.tensor_copy(od[:], accs[t][:])
                else:
                    nc.scalar.activation(od[:], accs[t][:], ACT_COPY)
                nt_ = ngrp * 8 + t
                eng = nc.sync if t % 2 == 0 else nc.gpsimd
                eng.dma_start(out[nt_ * 128:(nt_ + 1) * 128, :], od[:])


# ---------------------------------------------------------------------------
# host side
# ---------------------------------------------------------------------------

_DEINT = np.concatenate([np.arange(0, HD, 2), np.arange(1, HD, 2)])


def _bf16(a):
    return np.ascontiguousarray(a.astype(ml_dtypes.bfloat16))


def make_host_inputs(x, W_qkv, W_o):
    """Build the 8 per-core input dicts from the full problem inputs."""
    x = np.ascontiguousarray(np.asarray(x, dtype=np.float32))
    W_qkv = np.asarray(W_qkv, dtype=np.float32)
    W_o = np.asarray(W_o, dtype=np.float32)

    # RoPE tables, transposed + deinterleaved + duplicated/sign-folded
    theta = 1.0 / (ROPE_BASE ** (np.arange(0, HD, 2, dtype=np.float64) / HD))
    freqs = np.arange(N, dtype=np.float64)[:, None] * theta[None, :]  # [N, 64]
    cosT = np.cos(freqs).T.astype(np.float32)  # [64, N]
    sinT = np.sin(freqs).T.astype(np.float32)
    cc = np.concatenate([cosT, cosT], axis=0)  # [128, N]
    ss = np.concatenate([-sinT, sinT], axis=0)  # [128, N]

    # causal masks: M[i, t] = 1 if t >= i + 384 (shifted upper-tri family)
    i_idx = np.arange(128)[:, None]
    t_idx = np.arange(896)[None, :]
    mask = (t_idx >= i_idx + 384).astype(np.float32)

    ones_col = np.ones((128, 1), dtype=np.float32)
    ones_row = np.ones((1, 128), dtype=np.float32)

    # deinterleaved q/k weights: [D, H_TOT, HD]
    wq_full = W_qkv[:, 0 * D:1 * D].reshape(D, H_TOT, HD)[:, :, _DEINT]
    wk_full = W_qkv[:, 1 * D:2 * D].reshape(D, H_TOT, HD)[:, :, _DEINT]
    wv_full = W_qkv[:, 2 * D:3 * D]

    in_maps = []
    for c in range(N_CORES):
        b, g = divmod(c, 4)
        heads = slice(4 * g, 4 * g + 4)
        # wq/wk: [128, HL*KC*HD], block (hl*KC + k) is W[k*128:(k+1)*128,
        # head 4g+hl deint cols]
        wq_sel = wq_full[:, heads, :]  # [D, HL, HD]
        wk_sel = wk_full[:, heads, :]
        wq_pack = (wq_sel.reshape(KC, 128, HL, HD)
                   .transpose(1, 2, 0, 3).reshape(128, HL * KC * HD))
        wk_pack = (wk_sel.reshape(KC, 128, HL, HD)
                   .transpose(1, 2, 0, 3).reshape(128, HL * KC * HD))
        # wv: [128, KC*512], block k is Wv[k*128:(k+1)*128, 512g:512g+512]
        wv_pack = (wv_full[:, 512 * g:512 * g + 512]
                   .reshape(KC, 128, 512).transpose(1, 0, 2)
                   .reshape(128, KC * 512))
        # wo: [128, 16*512], chunk c2 = hl*4+r is W_o rows of global head
        # 4r+hl, columns 512g:512g+512
        wo_blocks = []
        for hl in range(HL):
            for r in range(4):
                gh = 4 * r + hl
                wo_blocks.append(
                    W_o[gh * 128:(gh + 1) * 128, 512 * g:512 * g + 512])
        wo_pack = (np.stack(wo_blocks, axis=0)  # [16, 128, 512]
                   .transpose(1, 0, 2).reshape(128, H_TOT * 512))
        in_maps.append({
            "xT": _bf16(x[b].T),
            "wq": _bf16(wq_pack),
            "wk": _bf16(wk_pack),
            "wv": _bf16(wv_pack),
            "wo": _bf16(wo_pack),
            "cc": _bf16(cc),
            "ss": _bf16(ss),
            "mask": _bf16(mask),
            "ones_col": _bf16(ones_col),
            "ones_row": ones_row,
        })
    return in_maps


def assemble_output(results):
    out = np.empty((B, N, D), dtype=np.float32)
    for c in range(N_CORES):
        b, g = divmod(c, 4)
        out[b, :, 512 * g:512 * g + 512] = results[c]["out"]
    return out


_PROGRAM = {}


def get_program(reps=1):
    if reps not in _PROGRAM:
        _PROGRAM[reps] = build_program(reps=reps)
    return _PROGRAM[reps]


def run(x, W_qkv, W_o, reps=1, **spmd_kwargs):
    nc = get_program(reps=reps)
    in_maps = make_host_inputs(x, W_qkv, W_o)
    res = run_bass_kernel_spmd(nc, in_maps, list(range(N_CORES)),
                               **spmd_kwargs)
    return assemble_output(res.results), res


def kernel(x, W_qkv, W_o):
    return run(x, W_qkv, W_o)[0]


if __name__ == "__main__":
    rng = np.random.default_rng(0)
    x = rng.standard_normal((B, N, D), dtype=np.float32)
    Wq = (rng.standard_normal((D, 3 * D), dtype=np.float32) * D ** -0.5)
    Wo = (rng.standard_normal((D, D), dtype=np.float32) * D ** -0.5)
    y = kernel(x, Wq, Wo)
    print("out:", y.shape, y.dtype, np.abs(y).max())


# revision 15
# speedup vs baseline: 1.2029x; 1.0173x over previous
"""Causal self-attention with RoPE for B=2, N=2048, D=2048, 16 heads,
distributed over 8 trn2 NeuronCores.

Sharding: core c = (b, g) with b = c // 4 (batch), g = c % 4 (head group of 4
heads).  Each core computes qkv projections + RoPE + causal attention for its
4 heads on its batch; per-head AllGathers ship each head's y^T to the other 3
cores of the batch while later heads still compute; each core then computes a
disjoint 512-column slice of the final o-projection.

v2 design notes (vs the fp32r two-half baseline):
- All matmul operands are bf16 (PSUM accumulation stays fp32).  bf16 enables
  FastWeightLoad (halves the per-matmul LDWEIGHTS cost that dominated the
  fp32r version), 2-4x DVE throughput, and half the DMA bytes.
- Stage A holds all 16 k-chunks of x^T in SBUF at once (bf16 makes it fit)
  and accumulates each qkv output tile across the full contraction in one
  PSUM bank: no half-pass SBUF adds, one PSUM->SBUF copy per tile, RoPE
  applied on the copy's output.
- Weights are host-packed so each logical stream is one large DMA.
- Softmax uses no max-subtraction (scores are O(1) by construction), masks
  multiply after exp, and row-normalization happens on y^T via a
  matmul-replicated reciprocal of the ones-matmul denominator.
"""

import numpy as np
from contextlib import ExitStack

import ml_dtypes

import concourse.bass as bass
import concourse.tile as tile
import concourse.mybir as mybir
from concourse.bass_utils import run_bass_kernel_spmd

F32 = mybir.dt.float32
F32R = mybir.dt.float32r
BF16 = mybir.dt.bfloat16

B = 2
N = 2048
D = 2048
H_TOT = 16
HD = 128  # head dim
HL = 4  # heads per core
N_CORES = 8
ROPE_BASE = 10000.0
INV_SQRT_HD = 1.0 / float(np.sqrt(HD))

NT = N // 512  # 4 n-tiles of 512
KC = D // 128  # 16 contraction chunks
ACT_COPY = mybir.ActivationFunctionType.Copy
ACT_EXP = mybir.ActivationFunctionType.Exp
ACT_LN = mybir.ActivationFunctionType.Ln


def split_multi_waits(nc, max_waits=1):
    """This container's walrus supports a single sync-wait per instruction;
    move extra waits onto preceding same-engine NoOps."""
    ctr = 0
    for f in nc.m.functions:
        for bb in f.blocks:
            new_list = []
            for inst in bb.instructions:
                si = inst.sync_info
                if si is not None and len(si.on_wait) > max_waits:
                    waits = list(si.on_wait)
                    for w in waits[:-max_waits]:
                        nop = mybir.InstNoOp(
                            name=f"antsplitw-{ctr}",
                            engine=inst.engine,
                            sync_info=mybir.SyncInfo(on_update=[], on_wait=[w]),
                        )
                        ctr += 1
                        new_list.append(nop)
                    si.on_wait = waits[-max_waits:]
                new_list.append(inst)
            bb.instructions[:] = new_list
    return ctr


def build_program(reps=1):
    nc = bass.Bass(num_devices=N_CORES)

    xT = nc.dram_tensor("xT", [D, N], BF16, kind="ExternalInput")
    wq = nc.dram_tensor("wq", [128, HL * KC * HD], BF16, kind="ExternalInput")
    wk = nc.dram_tensor("wk", [128, HL * KC * HD], BF16, kind="ExternalInput")
    wv = nc.dram_tensor("wv", [128, KC * 512], BF16, kind="ExternalInput")
    wo = nc.dram_tensor("wo", [128, H_TOT * 512], BF16, kind="ExternalInput")
    cc = nc.dram_tensor("cc", [128, N], BF16, kind="ExternalInput")
    ss = nc.dram_tensor("ss", [128, N], BF16, kind="ExternalInput")
    mask_in = nc.dram_tensor("mask", [128, 896], BF16, kind="ExternalInput")
    ones_col_in = nc.dram_tensor("ones_col", [128, 1], BF16, kind="ExternalInput")
    ones_row_in = nc.dram_tensor("ones_row", [1, 128], F32R, kind="ExternalInput")
    out = nc.dram_tensor("out", [N, 512], F32, kind="ExternalOutput")

    with nc.allow_low_precision(reason="bf16 matmul pipeline"):
        with tile.TileContext(nc) as tc:
            for rep in range(reps):
                _emit_rep(nc, tc, rep, xT, wq, wk, wv, wo, cc, ss, mask_in,
                          ones_col_in, ones_row_in, out)

    split_multi_waits(nc)
    return nc


def _emit_rep(nc, tc, rep, xT, wq, wk, wv, wo, cc, ss, mask_in,
              ones_col_in, ones_row_in, out):
    with ExitStack() as rep_ctx:
        const = rep_ctx.enter_context(tc.tile_pool(name=f"const{rep}", bufs=1))
        qk_pool = rep_ctx.enter_context(tc.tile_pool(name=f"qk{rep}", bufs=8))
        vn_pool = rep_ctx.enter_context(tc.tile_pool(name=f"vn{rep}", bufs=16))
        yn_pool = rep_ctx.enter_context(tc.tile_pool(name=f"yn{rep}", bufs=4))
        pt_pool = rep_ctx.enter_context(tc.tile_pool(name=f"pt{rep}", bufs=7))
        sm_pool = rep_ctx.enter_context(tc.tile_pool(name=f"sm{rep}", bufs=2))
        dram = rep_ctx.enter_context(
            tc.tile_pool(name=f"dram{rep}", bufs=1, space="DRAM"))

        mask_t = const.tile([128, 896], BF16, tag="mask")
        nc.gpsimd.dma_start(mask_t[:], mask_in[:])
        cc_t = const.tile([128, N], BF16, tag="cc")
        nc.gpsimd.dma_start(cc_t[:], cc[:])
        ss_t = const.tile([128, N], BF16, tag="ss")
        nc.gpsimd.dma_start(ss_t[:], ss[:])
        ones_col = const.tile([128, 1], BF16, tag="ones_col")
        nc.gpsimd.dma_start(ones_col[:], ones_col_in[:])
        ones_row = const.tile([1, 128], F32R, tag="ones_row")
        nc.gpsimd.dma_start(ones_row[:], ones_row_in[:])

        # head 3's bounce/gather is split into two column halves so its
        # AllGather can fire mid-B(3) and hide under stage C's first n-group
        y_bounce = [dram.tile([HD, N], BF16, name=f"yb{rep}_{h}")
                    for h in range(HL - 1)]
        y_gather = [dram.tile([4 * HD, N], BF16, name=f"yg{rep}_{h}")
                    for h in range(HL - 1)]
        yb3 = [dram.tile([HD, 512], BF16, name=f"yb3{rep}_{i}")
               for i in range(4)]
        yg3 = [dram.tile([4 * HD, 512], BF16, name=f"yg3{rep}_{i}")
               for i in range(4)]

        # persistent per-head q/k (RoPE'd, bf16) and v (natural layout)
        qr = [qk_pool.tile([128, N], BF16, tag="qr", name=f"qr{rep}_{h}")
              for h in range(HL)]
        kr = [qk_pool.tile([128, N], BF16, tag="kr", name=f"kr{rep}_{h}")
              for h in range(HL)]
        vn = [vn_pool.tile([128, 512], BF16, tag="vn", name=f"vn{rep}_{i}")
              for i in range(KC)]
        yn = [yn_pool.tile([128, 512], BF16, tag="yn", name=f"yn{rep}_{i}")
              for i in range(4 * NT)]

        def emit_ag(ins_t, outs_t):
            nc.gpsimd.collective_compute(
                "AllGather",
                mybir.AluOpType.bypass,
                replica_groups=[[0, 1, 2, 3], [4, 5, 6, 7]],
                ins=[ins_t.opt()],
                outs=[outs_t.opt()],
            )

        def emit_a_qk(hl):
            for wdram, dst in ((wq, qr), (wk, kr)):
                wt = w_pool.tile([128, KC * HD], BF16, tag="wqk")
                nc.sync.dma_start(
                    wt[:], wdram[:, hl * KC * HD:(hl + 1) * KC * HD])
                for jn in range(NT):
                    acc = psA.tile([128, 512], F32, tag="psA",
                                   name=f"psA{rep}_{hl}_{jn}")
                    for k in range(KC):
                        nc.tensor.matmul(
                            acc[:], wt[:, k * HD:(k + 1) * HD],
                            xh[k][:, jn * 512:(jn + 1) * 512],
                            start=(k == 0), stop=(k == KC - 1))
                    bs = slice(jn * 512, (jn + 1) * 512)
                    q0 = rope_pool.tile([128, 512], BF16, tag="q0", bufs=2)
                    nc.vector.tensor_copy(q0[:], acc[:])
                    sw = rope_pool.tile([128, 512], BF16, tag="sw", bufs=2)
                    nc.sync.dma_start(sw[0:64, :], q0[64:128, :])
                    nc.sync.dma_start(sw[64:128, :], q0[0:64, :])
                    t1 = rope_pool.tile([128, 512], BF16, tag="t1", bufs=2)
                    nc.vector.tensor_mul(t1[:], q0[:], cc_t[:, bs])
                    nc.vector.tensor_mul(sw[:], sw[:], ss_t[:, bs])
                    nc.vector.tensor_add(dst[hl][:, bs], t1[:], sw[:])

        def emit_b(hl):
            deferred = [None]
            for jn in range(NT):
                ims = list(range(min(16, 4 * jn + 4)))
                pts = []
                for im in ims:
                    s = psS.tile([128, 512], F32, tag="psS")
                    nc.tensor.matmul(
                        s[:], kr[hl][:, im * 128:(im + 1) * 128],
                        qr[hl][:, jn * 512:(jn + 1) * 512],
                        start=True, stop=True)
                    pt = pt_pool.tile([128, 512], BF16, tag="pt")
                    if im >= 4 * jn:
                        k2 = im - 4 * jn
                        pe = pt_pool.tile([128, 512], BF16, tag="pe", bufs=2)
                        nc.scalar.activation(pe[:], s[:], ACT_EXP,
                                             scale=INV_SQRT_HD)
                        nc.vector.tensor_mul(
                            pt[:], pe[:],
                            mask_t[:, (3 - k2) * 128:(3 - k2) * 128 + 512])
                    else:
                        nc.scalar.activation(pt[:], s[:], ACT_EXP,
                                             scale=INV_SQRT_HD)
                    pts.append(pt)
                y_acc = psY.tile([128, 512], F32, tag="psY")
                for idx, pt in enumerate(pts):
                    nc.tensor.matmul(
                        y_acc[:], vn[ims[idx]][:, hl * HD:(hl + 1) * HD],
                        pt[:], start=(idx == 0), stop=(idx == len(pts) - 1))
                dsum = []
                for i in range(0, len(pts) - 1, 2):
                    ds = pt_pool.tile([128, 512], BF16, tag="ds", bufs=2)
                    nc.vector.tensor_add(ds[:], pts[i][:], pts[i + 1][:])
                    dsum.append(ds)
                if len(pts) % 2:
                    dsum.append(pts[-1])
                d2 = []
                for i in range(0, len(dsum) - 1, 2):
                    ds = pt_pool.tile([128, 512], BF16, tag="ds2", bufs=2)
                    nc.vector.tensor_add(ds[:], dsum[i][:], dsum[i + 1][:])
                    d2.append(ds)
                if len(dsum) % 2:
                    d2.append(dsum[-1])
                dsum = d2
                den = psD.tile([1, 512], F32, tag="psD")
                for idx, ds in enumerate(dsum):
                    nc.tensor.matmul(
                        den[:], ones_col[:], ds[:],
                        start=(idx == 0), stop=(idx == len(dsum) - 1))
                den_inv = sm_pool.tile([1, 512], F32R, tag="den_inv")
                nc.scalar.activation(den_inv[:], den[:], ACT_LN)
                nc.scalar.activation(den_inv[:], den_inv[:], ACT_EXP,
                                     scale=-1.0)

                if deferred[0] is not None:
                    deferred[0]()
                    deferred[0] = None

                def _fin(jn=jn, y_acc=y_acc, den_inv=den_inv):
                    rep_ps = psR.tile([128, 512], F32, tag="psR")
                    nc.tensor.matmul(rep_ps[:], ones_row[:], den_inv[:],
                                     start=True, stop=True)
                    rinv = sm_pool.tile([128, 512], BF16, tag="rinv", bufs=2)
                    nc.scalar.activation(rinv[:], rep_ps[:], ACT_COPY)
                    ynt = yn[hl * NT + jn]
                    nc.vector.tensor_mul(ynt[:], y_acc[:], rinv[:])
                    if hl < HL - 1:
                        nc.gpsimd.dma_start(
                            y_bounce[hl][:, jn * 512:(jn + 1) * 512], ynt[:])
                    else:
                        nc.gpsimd.dma_start(yb3[jn][:], ynt[:])
                        emit_ag(yb3[jn], yg3[jn])

                if hl < HL - 1:
                    deferred[0] = _fin
                else:
                    _fin()  # keep head-3's quarter-AllGathers prompt
            if deferred[0] is not None:
                deferred[0]()
                deferred[0] = None
            if hl < HL - 1:
                emit_ag(y_bounce[hl], y_gather[hl])

        with ExitStack() as ctxB:
            psS = ctxB.enter_context(
                tc.tile_pool(name=f"psS{rep}", bufs=2, space="PSUM"))
            psY = ctxB.enter_context(
                tc.tile_pool(name=f"psY{rep}", bufs=2, space="PSUM"))
            psD = ctxB.enter_context(
                tc.tile_pool(name=f"psD{rep}", bufs=1, space="PSUM"))
            psR = ctxB.enter_context(
                tc.tile_pool(name=f"psR{rep}", bufs=1, space="PSUM"))

            with ExitStack() as ctx:
                xh_pool = ctx.enter_context(
                    tc.tile_pool(name=f"xh{rep}", bufs=16))
                w_pool = ctx.enter_context(tc.tile_pool(name=f"w{rep}", bufs=2))
                wv_pool = ctx.enter_context(
                    tc.tile_pool(name=f"wv{rep}", bufs=1))
                rope_pool = ctx.enter_context(
                    tc.tile_pool(name=f"rope{rep}", bufs=6))
                psA = ctx.enter_context(
                    tc.tile_pool(name=f"psA{rep}", bufs=2, space="PSUM"))

                # ---- loads ------------------------------------------------
                wv_t = wv_pool.tile([128, KC * 512], BF16, tag="wv")
                nc.gpsimd.dma_start(wv_t[:], wv[:])
                xh = []
                for k in range(KC):
                    xt = xh_pool.tile([128, N], BF16, tag="xh")
                    eng = nc.sync if k % 2 == 0 else nc.scalar
                    eng.dma_start(xt[:], xT[k * 128:(k + 1) * 128, :])
                    xh.append(xt)

                # ---- stage A-v: v projection (natural layout) -------------
                for nchunk in range(16):
                    vacc = psA.tile([128, 512], F32, tag="psA",
                                    name=f"psAv{rep}_{nchunk}")
                    for k in range(KC):
                        nc.tensor.matmul(
                            vacc[:],
                            xh[k][:, nchunk * 128:(nchunk + 1) * 128],
                            wv_t[:, k * 512:(k + 1) * 512],
                            start=(k == 0), stop=(k == KC - 1))
                    nc.vector.tensor_copy(vn[nchunk][:], vacc[:])

                # ---- stage A-qk + RoPE interleaved with stage B per head --
                for hl in range(HL):
                    emit_a_qk(hl)
                    if hl < HL - 1:
                        emit_b(hl)

            # ctx closed: xh/w/wv/rope SBUF freed -> stage C loads overlap
            # B(3).  These pools live until rep_ctx exits.
            wo_pool = rep_ctx.enter_context(
                tc.tile_pool(name=f"wo{rep}", bufs=1))
            yg_pool = rep_ctx.enter_context(
                tc.tile_pool(name=f"yg{rep}", bufs=12))
            od_pool = rep_ctx.enter_context(
                tc.tile_pool(name=f"od{rep}", bufs=4))
            wo_t = wo_pool.tile([128, H_TOT * 512], BF16, tag="wo")
            nc.sync.dma_start(wo_t[:], wo[:])
            ygs = [None] * 12
            for hl in range(HL - 1):
                for r in range(4):
                    yg = yg_pool.tile([128, N], BF16, tag="yg")
                    nc.sync.dma_start(
                        yg[:], y_gather[hl][r * 128:(r + 1) * 128, :])
                    ygs[hl * 4 + r] = yg

            emit_b(HL - 1)

        # ctxB closed: all 6 B-phase PSUM banks free for stage C
        psC = rep_ctx.enter_context(
            tc.tile_pool(name=f"psC{rep}", bufs=8, space="PSUM"))
        yg3s = [[None] * 4 for _ in range(4)]
        for q in range(4):
            for r in range(4):
                yg = yg_pool.tile([128, 512], BF16, tag="yg3", bufs=16)
                eng = nc.sync if r % 2 == 0 else nc.gpsimd
                eng.dma_start(
                    yg[:], yg3[q][r * 128:(r + 1) * 128, :])
                yg3s[q][r] = yg

        # ---- stage C: o-projection, chunk-outer so head-3 chunks are last -
        # chunk c2 = hl*4 + r maps to rows r*128 of y_gather[hl]
        for ngrp in range(2):
            accs = [psC.tile([128, 512], F32, tag="psC",
                             name=f"psC{rep}_{ngrp}_{i}")
                    for i in range(8)]
            c2_order = (list(range(16)) if ngrp == 0
                        else list(range(12, 16)) + list(range(12)))
            for ci, c2 in enumerate(c2_order):
                hl, r = divmod(c2, 4)
                for t in range(8):
                    nt_ = ngrp * 8 + t
                    if hl < HL - 1:
                        lhsT = ygs[c2][:, nt_ * 128:(nt_ + 1) * 128]
                    else:
                        lhsT = yg3s[nt_ // 4][r][:, (nt_ % 4) * 128:
                                                 (nt_ % 4) * 128 + 128]
                    nc.tensor.matmul(
                        accs[t][:], lhsT,
                        wo_t[:, c2 * 512:(c2 + 1) * 512],
                        start=(ci == 0), stop=(ci == 15))
            for t in range(8):
                od = od_pool.tile([128, 512], F32, tag="od")
                if t % 2 == 0:
                    nc.vector.tensor_copy(od[:], accs[t][:])
                else:
                    nc.scalar.activation(od[:], accs[t][:], ACT_COPY)
                nt_ = ngrp * 8 + t
                eng = nc.sync if t % 2 == 0 else nc.gpsimd
                eng.dma_start(out[nt_ * 128:(nt_ + 1) * 128, :], od[:])


# ---------------------------------------------------------------------------
# host side
# ---------------------------------------------------------------------------

_DEINT = np.concatenate([np.arange(0, HD, 2), np.arange(1, HD, 2)])


def _bf16(a):
    return np.ascontiguousarray(a.astype(ml_dtypes.bfloat16))


def make_host_inputs(x, W_qkv, W_o):
    """Build the 8 per-core input dicts from the full problem inputs."""
    x = np.ascontiguousarray(np.asarray(x, dtype=np.float32))
    W_qkv = np.asarray(W_qkv, dtype=np.float32)
    W_o = np.asarray(W_o, dtype=np.float32)

    # RoPE tables, transposed + deinterleaved + duplicated/sign-folded
    theta = 1.0 / (ROPE_BASE ** (np.arange(0, HD, 2, dtype=np.float64) / HD))
    freqs = np.arange(N, dtype=np.float64)[:, None] * theta[None, :]  # [N, 64]
    cosT = np.cos(freqs).T.astype(np.float32)  # [64, N]
    sinT = np.sin(freqs).T.astype(np.float32)
    cc = np.concatenate([cosT, cosT], axis=0)  # [128, N]
    ss = np.concatenate([-sinT, sinT], axis=0)  # [128, N]

    # causal masks: M[i, t] = 1 if t >= i + 384 (shifted upper-tri family)
    i_idx = np.arange(128)[:, None]
    t_idx = np.arange(896)[None, :]
    mask = (t_idx >= i_idx + 384).astype(np.float32)

    ones_col = np.ones((128, 1), dtype=np.float32)
    ones_row = np.ones((1, 128), dtype=np.float32)

    # deinterleaved q/k weights: [D, H_TOT, HD]
    wq_full = W_qkv[:, 0 * D:1 * D].reshape(D, H_TOT, HD)[:, :, _DEINT]
    wk_full = W_qkv[:, 1 * D:2 * D].reshape(D, H_TOT, HD)[:, :, _DEINT]
    wv_full = W_qkv[:, 2 * D:3 * D]

    in_maps = []
    for c in range(N_CORES):
        b, g = divmod(c, 4)
        heads = slice(4 * g, 4 * g + 4)
        # wq/wk: [128, HL*KC*HD], block (hl*KC + k) is W[k*128:(k+1)*128,
        # head 4g+hl deint cols]
        wq_sel = wq_full[:, heads, :]  # [D, HL, HD]
        wk_sel = wk_full[:, heads, :]
        wq_pack = (wq_sel.reshape(KC, 128, HL, HD)
                   .transpose(1, 2, 0, 3).reshape(128, HL * KC * HD))
        wk_pack = (wk_sel.reshape(KC, 128, HL, HD)
                   .transpose(1, 2, 0, 3).reshape(128, HL * KC * HD))
        # wv: [128, KC*512], block k is Wv[k*128:(k+1)*128, 512g:512g+512]
        wv_pack = (wv_full[:, 512 * g:512 * g + 512]
                   .reshape(KC, 128, 512).transpose(1, 0, 2)
                   .reshape(128, KC * 512))
        # wo: [128, 16*512], chunk c2 = hl*4+r is W_o rows of global head
        # 4r+hl, columns 512g:512g+512
        wo_blocks = []
        for hl in range(HL):
            for r in range(4):
                gh = 4 * r + hl
                wo_blocks.append(
                    W_o[gh * 128:(gh + 1) * 128, 512 * g:512 * g + 512])
        wo_pack = (np.stack(wo_blocks, axis=0)  # [16, 128, 512]
                   .transpose(1, 0, 2).reshape(128, H_TOT * 512))
        in_maps.append({
            "xT": _bf16(x[b].T),
            "wq": _bf16(wq_pack),
            "wk": _bf16(wk_pack),
            "wv": _bf16(wv_pack),
            "wo": _bf16(wo_pack),
            "cc": _bf16(cc),
            "ss": _bf16(ss),
            "mask": _bf16(mask),
            "ones_col": _bf16(ones_col),
            "ones_row": ones_row,
        })
    return in_maps


def assemble_output(results):
    out = np.empty((B, N, D), dtype=np.float32)
    for c in range(N_CORES):
        b, g = divmod(c, 4)
        out[b, :, 512 * g:512 * g + 512] = results[c]["out"]
    return out


_PROGRAM = {}


def get_program(reps=1):
    if reps not in _PROGRAM:
        _PROGRAM[reps] = build_program(reps=reps)
    return _PROGRAM[reps]


def run(x, W_qkv, W_o, reps=1, **spmd_kwargs):
    nc = get_program(reps=reps)
    in_maps = make_host_inputs(x, W_qkv, W_o)
    res = run_bass_kernel_spmd(nc, in_maps, list(range(N_CORES)),
                               **spmd_kwargs)
    return assemble_output(res.results), res


def kernel(x, W_qkv, W_o):
    return run(x, W_qkv, W_o)[0]


if __name__ == "__main__":
    rng = np.random.default_rng(0)
    x = rng.standard_normal((B, N, D), dtype=np.float32)
    Wq = (rng.standard_normal((D, 3 * D), dtype=np.float32) * D ** -0.5)
    Wo = (rng.standard_normal((D, D), dtype=np.float32) * D ** -0.5)
    y = kernel(x, Wq, Wo)
    print("out:", y.shape, y.dtype, np.abs(y).max())
